# revision 32
# baseline (speedup 1.0000x reference)
"""T5-style relative-position-bias attention on 8 TRN2 NeuronCores.

Full-input contract: kernel(**inputs) takes the unsharded tensors and
returns the full [2, 2048, 1024] output.

Sharding: 16 heads / 8 cores = 2 heads per core, both batches on every
core (data stays identical; only weight shards differ). Each core
computes its partial output projection (its heads' contribution to the
full [B, S, D] output); the host sums the 8 partials.
"""

import math
import sys

sys.path.insert(0, "/opt/trn_rl_repo")

import numpy as np
import ml_dtypes

BF16 = ml_dtypes.bfloat16

B, S, D, H, HD = 2, 2048, 1024, 16, 64
N_CORES = 8
HEADS_PER_CORE = H // N_CORES  # 2
SCALING = HD ** (-0.5)
NUM_BUCKETS = 32
MAX_DISTANCE = 128

# q-block = 512 columns of the (transposed) score tile; k-tile = 128 rows.
QB = 512
KT = 128
N_QB = S // QB  # 4
N_KT = S // KT  # 16
# near-diagonal offsets m = kt - 4*qb for which bias varies inside the tile
NEAR_MS = list(range(-1, 5))  # -1..4


def _bucket_np(d):
    """Port of reference._relative_position_bucket (bidirectional), float32."""
    nb = NUM_BUCKETS // 2  # 16
    rb = (d > 0).astype(np.int32) * nb
    ad = np.abs(d)
    max_exact = nb // 2  # 8
    is_small = ad < max_exact
    rp = np.maximum(ad, 1).astype(np.float32)
    ril = max_exact + (
        np.log(rp / np.float32(max_exact))
        / np.float32(math.log(MAX_DISTANCE / max_exact))
        * np.float32(nb - max_exact)
    ).astype(np.int32)
    ril = np.minimum(ril, nb - 1)
    return rb + np.where(is_small, ad, ril)


def _near_bucket_tables():
    """Bucket index tile [128, 512] for each near offset m (head-independent)."""
    tables = {}
    p = np.arange(KT)[:, None]
    j = np.arange(QB)[None, :]
    for m in NEAR_MS:
        d = KT * m + p - j  # d = k - q
        tables[m] = _bucket_np(d)
    return tables


_NEAR_BUCKETS = _near_bucket_tables()


def _prep_core_inputs(c, hidden_states, Wq, Wk, Wv, Wo, rel_emb, xt_bf16):
    rows = slice(128 * c, 128 * (c + 1))
    wqt = np.ascontiguousarray(Wq[rows].T.reshape(8, 128, 128)).astype(BF16)
    wkt = np.ascontiguousarray(Wk[rows].T.reshape(8, 128, 128)).astype(BF16)
    wvt = np.ascontiguousarray(Wv[rows].T.reshape(8, 128, 128)).astype(BF16)
    wot = np.ascontiguousarray(Wo[:, rows].T).astype(BF16)  # [128, 1024]

    # E tiles: exp(bias) for near-diagonal tiles; [2 heads, 6 offsets, 128, 512]
    etab = np.empty((HEADS_PER_CORE * len(NEAR_MS), KT, QB), dtype=BF16)
    bfar = np.empty((4,), dtype=np.float32)
    for hl in range(HEADS_PER_CORE):
        h = HEADS_PER_CORE * c + hl
        for mi, m in enumerate(NEAR_MS):
            etab[hl * len(NEAR_MS) + mi] = np.exp(
                rel_emb[_NEAR_BUCKETS[m], h].astype(np.float32)
            ).astype(BF16)
        bfar[2 * hl + 0] = rel_emb[15, h]  # far negative (k << q)
        bfar[2 * hl + 1] = rel_emb[31, h]  # far positive (k >> q)
    bfar_t = np.tile(bfar[None, :], (128, 1)).astype(np.float32)
    bfarexp_t = np.exp(bfar_t).astype(np.float32)

    return {
        "xt": xt_bf16,
        "wqt": wqt,
        "wkt": wkt,
        "wvt": wvt,
        "wot": wot,
        "etab": etab,
        "bfar": bfar_t,
        "bfarexp": bfarexp_t,
    }


_PROGRAM_CACHE = {}
DEBUG_DUMPS = False
BUILD_LEVEL = 6  # 1=proj 2=+scores/exp 3=+ctx 4=+norm 5=wo-mm 6=full


def _build_program():
    if "nc" in _PROGRAM_CACHE:
        return _PROGRAM_CACHE["nc"]

    from contextlib import ExitStack

    import concourse.bass as bass
    import concourse.tile as tile
    from concourse import bacc, mybir
    from concourse.masks import make_identity

    f32 = mybir.dt.float32
    bf16 = mybir.dt.bfloat16
    Exp = mybir.ActivationFunctionType.Exp

    nc = bacc.Bacc("TRN2", target_bir_lowering=False, debug=False,
                   num_devices=N_CORES)

    xt_d = nc.dram_tensor("xt", [B * 8, 128, S], bf16, kind="ExternalInput").ap()
    wqt_d = nc.dram_tensor("wqt", [8, 128, 128], bf16, kind="ExternalInput").ap()
    wkt_d = nc.dram_tensor("wkt", [8, 128, 128], bf16, kind="ExternalInput").ap()
    wvt_d = nc.dram_tensor("wvt", [8, 128, 128], bf16, kind="ExternalInput").ap()
    wot_d = nc.dram_tensor("wot", [128, 1024], bf16, kind="ExternalInput").ap()
    etab_d = nc.dram_tensor("etab", [12, 128, 512], bf16, kind="ExternalInput").ap()
    bfar_d = nc.dram_tensor("bfar", [128, 4], f32, kind="ExternalInput").ap()
    bfarexp_d = nc.dram_tensor("bfarexp", [128, 4], f32,
                               kind="ExternalInput").ap()
    out_d = nc.dram_tensor("out", [B, S, D], f32, kind="ExternalOutput").ap()
    if DEBUG_DUMPS:
        dbg_qt = nc.dram_tensor("dbg_qt", [128, B * S], bf16,
                                kind="ExternalOutput").ap()
        dbg_kt = nc.dram_tensor("dbg_kt", [128, B * S], bf16,
                                kind="ExternalOutput").ap()
        dbg_v = nc.dram_tensor("dbg_v", [128, B * 16 * 130], bf16,
                               kind="ExternalOutput").ap()
        dbg_ut = nc.dram_tensor("dbg_ut", [128, 1024], bf16,
                                kind="ExternalOutput").ap()
        dbg_sct = nc.dram_tensor("dbg_sct", [128, 1024], f32,
                                 kind="ExternalOutput").ap()
        dbg_ctx = nc.dram_tensor("dbg_ctx", [2, 65, 512], f32,
                                 kind="ExternalOutput").ap()
        dbg_lct = nc.dram_tensor("dbg_lct", [2, 64, 512], bf16,
                                 kind="ExternalOutput").ap()
        dbg_rzb = nc.dram_tensor("dbg_rzb", [2, 64, 512], f32,
                                 kind="ExternalOutput").ap()

    VSLOT = 2 * 65  # [VA | 1 | VB | 1] per (b, kt)

    with tile.TileContext(nc) as tc, ExitStack() as ctx:
        const = ctx.enter_context(tc.tile_pool(name="const", bufs=1))

        xt_sb = const.tile([128, B * 8 * S], bf16, tag="xt")
        for i in range(B * 8):
            nc.sync.dma_start(xt_sb[:, S * i : S * (i + 1)], xt_d[i])
        wq_sb = const.tile([128, 8 * 128], bf16, tag="wq")
        wk_sb = const.tile([128, 8 * 128], bf16, tag="wk")
        wv_sb = const.tile([128, 8 * 128], bf16, tag="wv")
        for w_sb, w_d in ((wq_sb, wqt_d), (wk_sb, wkt_d), (wv_sb, wvt_d)):
            for i in range(8):
                nc.sync.dma_start(w_sb[:, 128 * i : 128 * (i + 1)], w_d[i])
        wot_sb = const.tile([128, 1024], bf16, tag="wot")
        nc.sync.dma_start(wot_sb[:], wot_d[:])
        etab_sb = const.tile([128, 12 * 512], bf16, tag="etab")
        for i in range(12):
            nc.sync.dma_start(etab_sb[:, 512 * i : 512 * (i + 1)], etab_d[i])
        bfar_sb = const.tile([128, 4], f32, tag="bfar")
        nc.sync.dma_start(bfar_sb[:], bfar_d[:])
        bfarexp_sb = const.tile([128, 4], f32, tag="bfarexp")
        nc.sync.dma_start(bfarexp_sb[:], bfarexp_d[:])
        ident = const.tile([128, 128], bf16, tag="ident")
        make_identity(nc, ident[:])

        qt_sb = const.tile([128, B * S], bf16, tag="qt")
        vt_sb = const.tile([128, B * S], bf16, tag="vtt")
        kt_sb = const.tile([128, B * S], bf16, tag="kt")
        v_sb = const.tile([128, B * N_KT * VSLOT], bf16, tag="v")
        for b in range(B):
            for kt in range(N_KT):
                base = (b * N_KT + kt) * VSLOT
                nc.gpsimd.memset(v_sb[:, base + 64 : base + 65], 1.0)
                nc.gpsimd.memset(v_sb[:, base + 129 : base + 130], 1.0)

        # ---- Phase B: projections ----
        # Pure dense matmul chains (Q/K/V all transposed, weight stationary,
        # N=512). The V transposes to natural layout happen inside phase C's
        # qb0 score stream so the PE never idles from HAM's point of view.
        with tc.tile_pool(name="projp", bufs=3, space="PSUM") as projp:
            for b in range(B):
                for qb in range(N_QB):
                    for wi, (w_sb, dst) in enumerate(((wq_sb, qt_sb),
                                                     (wk_sb, kt_sb),
                                                     (wv_sb, vt_sb))):
                        ps = projp.tile([128, 512], f32, tag="proj",
                                        name=f"pj_{b}_{qb}_{wi}")
                        for dt in range(8):
                            nc.tensor.matmul(
                                ps[:],
                                lhsT=w_sb[:, 128 * dt : 128 * (dt + 1)],
                                rhs=xt_sb[:, (b * 8 + dt) * S + qb * QB :
                                          (b * 8 + dt) * S + qb * QB + QB],
                                start=(dt == 0), stop=(dt == 7),
                            )
                        nc.vector.tensor_copy(
                            dst[:, b * S + qb * QB : b * S + qb * QB + QB], ps[:]
                        )

        # ---- Phase C+D: attention + output projection ----
        # Both batches interleaved at the kp level so the PE always has an
        # independent score-matmul stream while ACT chews on exp tiles.
        # PSUM budget: sct 2x2 banks + ctx 4x1 banks = 8.
        def cls(m):
            if m <= -2:
                return 0  # far negative
            if m >= 5:
                return 1  # far positive
            return 2  # near

        with tc.tile_pool(name="scp", bufs=2, space="PSUM") as scp, \
             tc.tile_pool(name="ctxp", bufs=1, space="PSUM") as ctxp, \
             tc.tile_pool(name="utp", bufs=8) as utp, \
             tc.tile_pool(name="ostg", bufs=4) as ostg, \
             tc.tile_pool(name="nrm", bufs=1) as nrm:

            def emit_scores_ctx(qb, ctxs):
                for kp in range(8):
                    m0 = 2 * kp - 4 * qb
                    m1 = m0 + 1
                    c0, c1 = cls(m0), cls(m1)
                    for b in range(B):
                        for hl in range(2):
                            sct = scp.tile([128, 1024], f32, tag="sc",
                                           name=f"sc_{qb}_{kp}_{b}_{hl}")
                            for half in range(2):
                                kt = 2 * kp + half
                                nc.tensor.matmul(
                                    sct[:, 512 * half : 512 * (half + 1)],
                                    lhsT=kt_sb[64 * hl : 64 * (hl + 1),
                                               b * S + kt * KT : b * S + kt * KT + KT],
                                    rhs=qt_sb[64 * hl : 64 * (hl + 1),
                                              b * S + qb * QB : b * S + qb * QB + QB],
                                    start=True, stop=True,
                                )
                            ut = utp.tile([128, 1024], bf16, tag="ut",
                                          name=f"ut_{qb}_{kp}_{b}_{hl}")
                            srcs = [(ut, 0), (ut, 512)]
                            if c0 == c1 and c0 != 2:
                                nc.scalar.activation(
                                    ut[:], sct[:], Exp,
                                    bias=bfar_sb[:, 2 * hl + c0 : 2 * hl + c0 + 1],
                                    scale=SCALING,
                                )
                            else:
                                nc.scalar.activation(
                                    ut[:], sct[:], Exp, bias=0.0, scale=SCALING
                                )
                                ut2 = utp.tile([128, 1024], bf16, tag="ut2",
                                               name=f"ut2_{qb}_{kp}_{b}_{hl}")
                                if c0 == c1 == 2:
                                    ei = (hl * 6 + (m0 + 1)) * 512
                                    nc.vector.tensor_mul(
                                        ut2[:], ut[:],
                                        etab_sb[:, ei : ei + 1024]
                                    )
                                    srcs = [(ut2, 0), (ut2, 512)]
                                else:
                                    for half, (m, cc) in enumerate(((m0, c0),
                                                                   (m1, c1))):
                                        usl = ut[:, 512 * half : 512 * (half + 1)]
                                        osl = ut2[:, 512 * half : 512 * (half + 1)]
                                        if cc == 2:
                                            ei = (hl * 6 + (m + 1)) * 512
                                            nc.vector.tensor_mul(
                                                osl, usl,
                                                etab_sb[:, ei : ei + 512]
                                            )
                                        else:
                                            col = 2 * hl + cc
                                            nc.vector.tensor_scalar_mul(
                                                osl, usl,
                                                bfarexp_sb[:, col : col + 1],
                                            )
                                        srcs[half] = (ut2, 512 * half)
                            if qb == 0 and hl == 0:
                                for half in range(2):
                                    kt = 2 * kp + half
                                    tr = scp.tile([128, 128], bf16, tag="sc",
                                                  name=f"vtr_{b}_{kt}")
                                    nc.tensor.transpose(
                                        tr[:],
                                        vt_sb[:, b * S + kt * KT :
                                              b * S + kt * KT + KT],
                                        ident[:],
                                    )
                                    base = (b * N_KT + kt) * VSLOT
                                    nc.vector.tensor_copy(
                                        v_sb[:, base : base + 64], tr[:, 0:64])
                                    nc.vector.tensor_copy(
                                        v_sb[:, base + 65 : base + 129],
                                        tr[:, 64:128])
                            for half in range(2):
                                kt = 2 * kp + half
                                base = (b * N_KT + kt) * VSLOT + 65 * hl
                                stile, soff = srcs[half]
                                nc.tensor.matmul(
                                    ctxs[(b, hl)][:],
                                    lhsT=v_sb[:, base : base + 65],
                                    rhs=stile[:, soff : soff + 512],
                                    start=(kt == 0), stop=(kt == N_KT - 1),
                                )

            def emit_norm_wo(qb, ctxs):
                # normalize: lct = ctx[0:64] / Z  (Z = row 64)
                # lct holds both heads: rows 0-63 = head A, 64-127 = head B
                # (head B written via DVE cross-quadrant 64-part op)
                lcts = {}
                for b in range(B):
                    lcts[b] = nrm.tile([128, 512], bf16, tag=f"lct{b}",
                                       name=f"lct{b}_{qb}", bufs=2)
                    for hl in range(2):
                        # custom DVE ops (approx recip, partition_broadcast)
                        # ignore AP partition offsets on HW: move the Z row
                        # to partition 0 with a native copy first, then keep
                        # everything at base partition 0.
                        rz = nrm.tile([128, 512], f32, tag=f"rz{b}{hl}",
                                      name=f"rz{b}{hl}_{qb}")
                        nc.vector.tensor_copy(rz[0:1, :],
                                              ctxs[(b, hl)][64:65, :])
                        rzf = nrm.tile([128, 512], f32, tag=f"rzf{b}{hl}",
                                       name=f"rzf{b}{hl}_{qb}")
                        nc.vector.reciprocal_approx_fast(
                            out=rzf[0:1, :], in_=rz[0:1, :]
                        )
                        rzb = nrm.tile([64, 512], f32, tag=f"rzb{b}{hl}",
                                       name=f"rzb{b}{hl}_{qb}")
                        nc.gpsimd.partition_broadcast(
                            rzb[:], rzf[0:1, :], channels=64
                        )
                        nc.vector.tensor_mul(
                            lcts[b][64 * hl : 64 * (hl + 1), :],
                            ctxs[(b, hl)][0:64, :], rzb[:])
                # output projection (2 accumulating K=64 matmuls per n-half)
                for b in range(B):
                    for st in range(4):
                        wo_ps = scp.tile([128, 1024], f32, tag="sc",
                                         name=f"wo_{qb}_{b}_{st}")
                        for nh in range(2):
                            nc.tensor.matmul(
                                wo_ps[:, nh * 512 : (nh + 1) * 512],
                                lhsT=lcts[b][:, st * 128 : (st + 1) * 128],
                                rhs=wot_sb[:, nh * 512 : (nh + 1) * 512],
                                start=True, stop=True,
                            )
                        ot = ostg.tile([128, 1024], f32, tag="ot",
                                       name=f"ot_{qb}_{b}_{st}")
                        nc.vector.tensor_copy(ot[:], wo_ps[:])
                        srow = qb * QB + st * 128
                        nc.gpsimd.dma_start(
                            out_d[b, srow : srow + 128, :], ot[:]
                        )

            prev = None
            for qb in range(N_QB):
                ctxs = {}
                for b in range(B):
                    for hl in range(2):
                        ctxs[(b, hl)] = ctxp.tile(
                            [65, 512], f32, tag=f"ctx{b}{hl}",
                            name=f"ctx{b}{hl}_{qb}")
                emit_scores_ctx(qb, ctxs)
                if prev is not None:
                    emit_norm_wo(*prev)
                prev = (qb, ctxs)
            emit_norm_wo(*prev)

        if DEBUG_DUMPS:
            nc.sync.dma_start(dbg_qt[:], qt_sb[:])
            nc.sync.dma_start(dbg_kt[:], kt_sb[:])
            nc.sync.dma_start(dbg_v[:], v_sb[:])

    nc.compile()
    _PROGRAM_CACHE["nc"] = nc
    return nc


def run(inputs, trace=False, trace_kwargs=None):
    """Returns (full_output, BassKernelResults)."""
    from concourse.bass_utils import run_bass_kernel_spmd

    hidden_states = np.asarray(inputs["hidden_states"], dtype=np.float32)
    Wq = np.asarray(inputs["Wq"], dtype=np.float32)
    Wk = np.asarray(inputs["Wk"], dtype=np.float32)
    Wv = np.asarray(inputs["Wv"], dtype=np.float32)
    Wo = np.asarray(inputs["Wo"], dtype=np.float32)
    rel_emb = np.asarray(inputs["rel_emb"], dtype=np.float32)

    xt = np.ascontiguousarray(hidden_states.transpose(0, 2, 1))  # [B, D, S]
    xt_bf16 = np.ascontiguousarray(xt.reshape(B * 8, 128, S)).astype(BF16)

    nc = _build_program()
    in_maps = [
        _prep_core_inputs(c, hidden_states, Wq, Wk, Wv, Wo, rel_emb, xt_bf16)
        for c in range(N_CORES)
    ]
    res = run_bass_kernel_spmd(
        nc, in_maps, list(range(N_CORES)), trace=trace,
        **(trace_kwargs or {}),
    )
    out = np.zeros((B, S, D), dtype=np.float32)
    for c in range(N_CORES):
        out += res.results[c]["out"]
    return out, res


def kernel(**inputs):
    out, _ = run(inputs)
    return out


# revision 33
# speedup vs baseline: 1.0544x; 1.0544x over previous
"""T5-style relative-position-bias attention on 8 TRN2 NeuronCores.

Full-input contract: kernel(**inputs) takes the unsharded tensors and
returns the full [2, 2048, 1024] output.

Sharding: 16 heads / 8 cores = 2 heads per core, both batches on every
core (data stays identical; only weight shards differ). Each core
computes its partial output projection (its heads' contribution to the
full [B, S, D] output); the host sums the 8 partials.
"""

import math
import sys

sys.path.insert(0, "/opt/trn_rl_repo")

import numpy as np
import ml_dtypes

BF16 = ml_dtypes.bfloat16

B, S, D, H, HD = 2, 2048, 1024, 16, 64
N_CORES = 8
HEADS_PER_CORE = H // N_CORES  # 2
SCALING = HD ** (-0.5)
NUM_BUCKETS = 32
MAX_DISTANCE = 128

# q-block = 512 columns of the (transposed) score tile; k-tile = 128 rows.
QB = 512
KT = 128
N_QB = S // QB  # 4
N_KT = S // KT  # 16
# near-diagonal offsets m = kt - 4*qb for which bias varies inside the tile
NEAR_MS = list(range(-1, 5))  # -1..4


def _bucket_np(d):
    """Port of reference._relative_position_bucket (bidirectional), float32."""
    nb = NUM_BUCKETS // 2  # 16
    rb = (d > 0).astype(np.int32) * nb
    ad = np.abs(d)
    max_exact = nb // 2  # 8
    is_small = ad < max_exact
    rp = np.maximum(ad, 1).astype(np.float32)
    ril = max_exact + (
        np.log(rp / np.float32(max_exact))
        / np.float32(math.log(MAX_DISTANCE / max_exact))
        * np.float32(nb - max_exact)
    ).astype(np.int32)
    ril = np.minimum(ril, nb - 1)
    return rb + np.where(is_small, ad, ril)


def _near_bucket_tables():
    """Bucket index tile [128, 512] for each near offset m (head-independent)."""
    tables = {}
    p = np.arange(KT)[:, None]
    j = np.arange(QB)[None, :]
    for m in NEAR_MS:
        d = KT * m + p - j  # d = k - q
        tables[m] = _bucket_np(d)
    return tables


_NEAR_BUCKETS = _near_bucket_tables()


def _prep_core_inputs(c, hidden_states, Wq, Wk, Wv, Wo, rel_emb, xt_bf16):
    rows = slice(128 * c, 128 * (c + 1))
    wqt = np.ascontiguousarray(Wq[rows].T.reshape(8, 128, 128)).astype(BF16)
    wkt = np.ascontiguousarray(Wk[rows].T.reshape(8, 128, 128)).astype(BF16)
    wvt = np.ascontiguousarray(Wv[rows].T.reshape(8, 128, 128)).astype(BF16)
    wot = np.ascontiguousarray(Wo[:, rows].T).astype(BF16)  # [128, 1024]

    # E tiles: exp(bias) for near-diagonal tiles; [2 heads, 6 offsets, 128, 512]
    etab = np.empty((HEADS_PER_CORE * len(NEAR_MS), KT, QB), dtype=BF16)
    bfar = np.empty((4,), dtype=np.float32)
    for hl in range(HEADS_PER_CORE):
        h = HEADS_PER_CORE * c + hl
        for mi, m in enumerate(NEAR_MS):
            etab[hl * len(NEAR_MS) + mi] = np.exp(
                rel_emb[_NEAR_BUCKETS[m], h].astype(np.float32)
            ).astype(BF16)
        bfar[2 * hl + 0] = rel_emb[15, h]  # far negative (k << q)
        bfar[2 * hl + 1] = rel_emb[31, h]  # far positive (k >> q)
    bfar_t = np.tile(bfar[None, :], (128, 1)).astype(np.float32)
    bfarexp_t = np.exp(bfar_t).astype(np.float32)

    return {
        "xt": xt_bf16,
        "wqt": wqt,
        "wkt": wkt,
        "wvt": wvt,
        "wot": wot,
        "etab": etab,
        "bfar": bfar_t,
        "bfarexp": bfarexp_t,
    }


_PROGRAM_CACHE = {}
DEBUG_DUMPS = False
BUILD_LEVEL = 6  # 1=proj 2=+scores/exp 3=+ctx 4=+norm 5=wo-mm 6=full


def _build_program():
    if "nc" in _PROGRAM_CACHE:
        return _PROGRAM_CACHE["nc"]

    from contextlib import ExitStack

    import concourse.bass as bass
    import concourse.tile as tile
    from concourse import bacc, mybir
    from concourse.masks import make_identity

    f32 = mybir.dt.float32
    bf16 = mybir.dt.bfloat16
    Exp = mybir.ActivationFunctionType.Exp

    nc = bacc.Bacc("TRN2", target_bir_lowering=False, debug=False,
                   num_devices=N_CORES)

    xt_d = nc.dram_tensor("xt", [B * 8, 128, S], bf16, kind="ExternalInput").ap()
    wqt_d = nc.dram_tensor("wqt", [8, 128, 128], bf16, kind="ExternalInput").ap()
    wkt_d = nc.dram_tensor("wkt", [8, 128, 128], bf16, kind="ExternalInput").ap()
    wvt_d = nc.dram_tensor("wvt", [8, 128, 128], bf16, kind="ExternalInput").ap()
    wot_d = nc.dram_tensor("wot", [128, 1024], bf16, kind="ExternalInput").ap()
    etab_d = nc.dram_tensor("etab", [12, 128, 512], bf16, kind="ExternalInput").ap()
    bfar_d = nc.dram_tensor("bfar", [128, 4], f32, kind="ExternalInput").ap()
    bfarexp_d = nc.dram_tensor("bfarexp", [128, 4], f32,
                               kind="ExternalInput").ap()
    out_d = nc.dram_tensor("out", [B, S, D], f32, kind="ExternalOutput").ap()
    if DEBUG_DUMPS:
        dbg_qt = nc.dram_tensor("dbg_qt", [128, B * S], bf16,
                                kind="ExternalOutput").ap()
        dbg_kt = nc.dram_tensor("dbg_kt", [128, B * S], bf16,
                                kind="ExternalOutput").ap()
        dbg_v = nc.dram_tensor("dbg_v", [128, B * 16 * 130], bf16,
                               kind="ExternalOutput").ap()
        dbg_ut = nc.dram_tensor("dbg_ut", [128, 1024], bf16,
                                kind="ExternalOutput").ap()
        dbg_sct = nc.dram_tensor("dbg_sct", [128, 1024], f32,
                                 kind="ExternalOutput").ap()
        dbg_ctx = nc.dram_tensor("dbg_ctx", [2, 65, 512], f32,
                                 kind="ExternalOutput").ap()
        dbg_lct = nc.dram_tensor("dbg_lct", [2, 64, 512], bf16,
                                 kind="ExternalOutput").ap()
        dbg_rzb = nc.dram_tensor("dbg_rzb", [2, 64, 512], f32,
                                 kind="ExternalOutput").ap()

    VSLOT = 2 * 65  # [VA | 1 | VB | 1] per (b, kt)

    with tile.TileContext(nc) as tc, ExitStack() as ctx:
        const = ctx.enter_context(tc.tile_pool(name="const", bufs=1))

        xt_sb = const.tile([128, B * 8 * S], bf16, tag="xt")
        for i in range(B * 8):
            nc.sync.dma_start(xt_sb[:, S * i : S * (i + 1)], xt_d[i])
        wq_sb = const.tile([128, 8 * 128], bf16, tag="wq")
        wk_sb = const.tile([128, 8 * 128], bf16, tag="wk")
        wv_sb = const.tile([128, 8 * 128], bf16, tag="wv")
        for w_sb, w_d in ((wq_sb, wqt_d), (wk_sb, wkt_d), (wv_sb, wvt_d)):
            for i in range(8):
                nc.sync.dma_start(w_sb[:, 128 * i : 128 * (i + 1)], w_d[i])
        wot_sb = const.tile([128, 1024], bf16, tag="wot")
        nc.sync.dma_start(wot_sb[:], wot_d[:])
        etab_sb = const.tile([128, 12 * 512], bf16, tag="etab")
        for i in range(12):
            nc.sync.dma_start(etab_sb[:, 512 * i : 512 * (i + 1)], etab_d[i])
        bfar_sb = const.tile([128, 4], f32, tag="bfar")
        nc.sync.dma_start(bfar_sb[:], bfar_d[:])
        bfarexp_sb = const.tile([128, 4], f32, tag="bfarexp")
        nc.sync.dma_start(bfarexp_sb[:], bfarexp_d[:])
        ident = const.tile([128, 128], bf16, tag="ident")
        make_identity(nc, ident[:])

        qt_sb = const.tile([128, B * S], bf16, tag="qt")
        vt_sb = const.tile([128, B * S], bf16, tag="vtt")
        kt_sb = const.tile([128, B * S], bf16, tag="kt")
        v_sb = const.tile([128, B * N_KT * VSLOT], bf16, tag="v")
        for b in range(B):
            for kt in range(N_KT):
                base = (b * N_KT + kt) * VSLOT
                nc.gpsimd.memset(v_sb[:, base + 64 : base + 65], 1.0)
                nc.gpsimd.memset(v_sb[:, base + 129 : base + 130], 1.0)

        # ---- Phases B+C staggered ----
        # The PE and ACT have nearly equal total work, but projections are
        # PE-only while attention is ACT-paced. Staggering batch 1's
        # projections (and V transposes) into batch 0's attention keeps the
        # PE dense so the HAM clock gate stays at full rate.
        def cls(m):
            if m <= -2:
                return 0  # far negative
            if m >= 5:
                return 1  # far positive
            return 2  # near

        with tc.tile_pool(name="scp", bufs=2, space="PSUM") as scp, \
             tc.tile_pool(name="ctxp", bufs=1, space="PSUM") as ctxp, \
             tc.tile_pool(name="utp", bufs=8) as utp, \
             tc.tile_pool(name="ostg", bufs=4) as ostg, \
             tc.tile_pool(name="nrm", bufs=1) as nrm:

            def emit_proj_chain(b, qb, wi):
                w_sb, dst = ((wq_sb, qt_sb), (wk_sb, kt_sb),
                             (wv_sb, vt_sb))[wi]
                ps = scp.tile([128, 512], f32, tag="sc",
                              name=f"pj_{b}_{qb}_{wi}")
                for dt in range(8):
                    nc.tensor.matmul(
                        ps[:],
                        lhsT=w_sb[:, 128 * dt : 128 * (dt + 1)],
                        rhs=xt_sb[:, (b * 8 + dt) * S + qb * QB :
                                  (b * 8 + dt) * S + qb * QB + QB],
                        start=(dt == 0), stop=(dt == 7),
                    )
                nc.vector.tensor_copy(
                    dst[:, b * S + qb * QB : b * S + qb * QB + QB], ps[:]
                )

            def emit_vtrans(b, kt):
                tr = scp.tile([128, 128], bf16, tag="sc",
                              name=f"vtr_{b}_{kt}")
                nc.tensor.transpose(
                    tr[:], vt_sb[:, b * S + kt * KT : b * S + kt * KT + KT],
                    ident[:],
                )
                base = (b * N_KT + kt) * VSLOT
                nc.vector.tensor_copy(v_sb[:, base : base + 64], tr[:, 0:64])
                nc.vector.tensor_copy(v_sb[:, base + 65 : base + 129],
                                      tr[:, 64:128])

            def emit_stream_kp(kp, b, qb, ctxs):
                m0 = 2 * kp - 4 * qb
                m1 = m0 + 1
                c0, c1 = cls(m0), cls(m1)
                for hl in range(2):
                    sct = scp.tile([128, 1024], f32, tag="sc",
                                   name=f"sc_{qb}_{kp}_{b}_{hl}")
                    for half in range(2):
                        kt = 2 * kp + half
                        nc.tensor.matmul(
                            sct[:, 512 * half : 512 * (half + 1)],
                            lhsT=kt_sb[64 * hl : 64 * (hl + 1),
                                       b * S + kt * KT : b * S + kt * KT + KT],
                            rhs=qt_sb[64 * hl : 64 * (hl + 1),
                                      b * S + qb * QB : b * S + qb * QB + QB],
                            start=True, stop=True,
                        )
                    ut = utp.tile([128, 1024], bf16, tag="ut",
                                  name=f"ut_{qb}_{kp}_{b}_{hl}")
                    srcs = [(ut, 0), (ut, 512)]
                    if c0 == c1 and c0 != 2:
                        nc.scalar.activation(
                            ut[:], sct[:], Exp,
                            bias=bfar_sb[:, 2 * hl + c0 : 2 * hl + c0 + 1],
                            scale=SCALING,
                        )
                    else:
                        nc.scalar.activation(
                            ut[:], sct[:], Exp, bias=0.0, scale=SCALING
                        )
                        ut2 = utp.tile([128, 1024], bf16, tag="ut2",
                                       name=f"ut2_{qb}_{kp}_{b}_{hl}")
                        if c0 == c1 == 2:
                            ei = (hl * 6 + (m0 + 1)) * 512
                            nc.vector.tensor_mul(
                                ut2[:], ut[:], etab_sb[:, ei : ei + 1024]
                            )
                            srcs = [(ut2, 0), (ut2, 512)]
                        else:
                            for half, (m, cc) in enumerate(((m0, c0),
                                                           (m1, c1))):
                                usl = ut[:, 512 * half : 512 * (half + 1)]
                                osl = ut2[:, 512 * half : 512 * (half + 1)]
                                if cc == 2:
                                    ei = (hl * 6 + (m + 1)) * 512
                                    nc.vector.tensor_mul(
                                        osl, usl, etab_sb[:, ei : ei + 512]
                                    )
                                else:
                                    col = 2 * hl + cc
                                    nc.vector.tensor_scalar_mul(
                                        osl, usl, bfarexp_sb[:, col : col + 1]
                                    )
                                srcs[half] = (ut2, 512 * half)
                    for half in range(2):
                        kt = 2 * kp + half
                        base = (b * N_KT + kt) * VSLOT + 65 * hl
                        stile, soff = srcs[half]
                        nc.tensor.matmul(
                            ctxs[(b, hl)][:],
                            lhsT=v_sb[:, base : base + 65],
                            rhs=stile[:, soff : soff + 512],
                            start=(kt == 0), stop=(kt == N_KT - 1),
                        )

            def emit_norm_wo(b, qb, ctxs):
                lct = nrm.tile([128, 512], bf16, tag=f"lct{b}",
                               name=f"lct{b}_{qb}", bufs=2)
                for hl in range(2):
                    rz = nrm.tile([128, 512], f32, tag=f"rz{b}{hl}",
                                  name=f"rz{b}{hl}_{qb}")
                    nc.vector.tensor_copy(rz[0:1, :], ctxs[(b, hl)][64:65, :])
                    rzf = nrm.tile([128, 512], f32, tag=f"rzf{b}{hl}",
                                   name=f"rzf{b}{hl}_{qb}")
                    nc.vector.reciprocal_approx_fast(
                        out=rzf[0:1, :], in_=rz[0:1, :]
                    )
                    rzb = nrm.tile([64, 512], f32, tag=f"rzb{b}{hl}",
                                   name=f"rzb{b}{hl}_{qb}")
                    nc.gpsimd.partition_broadcast(
                        rzb[:], rzf[0:1, :], channels=64
                    )
                    nc.vector.tensor_mul(
                        lct[64 * hl : 64 * (hl + 1), :],
                        ctxs[(b, hl)][0:64, :], rzb[:])
                for st in range(4):
                    wo_ps = scp.tile([128, 1024], f32, tag="sc",
                                     name=f"wo_{qb}_{b}_{st}")
                    for nh in range(2):
                        nc.tensor.matmul(
                            wo_ps[:, nh * 512 : (nh + 1) * 512],
                            lhsT=lct[:, st * 128 : (st + 1) * 128],
                            rhs=wot_sb[:, nh * 512 : (nh + 1) * 512],
                            start=True, stop=True,
                        )
                    ot = ostg.tile([128, 1024], f32, tag="ot",
                                   name=f"ot_{qb}_{b}_{st}")
                    nc.vector.tensor_copy(ot[:], wo_ps[:])
                    srow = qb * QB + st * 128
                    nc.gpsimd.dma_start(out_d[b, srow : srow + 128, :], ot[:])

            # batch-0 projections + transposes up front
            for qb in range(N_QB):
                for wi in range(3):
                    emit_proj_chain(0, qb, wi)
                for kt in range(4 * qb, 4 * qb + 4):
                    emit_vtrans(0, kt)

            # filler work for step 0: batch-1 projections + transposes
            filler = []
            for qb in range(N_QB):
                for wi in range(3):
                    filler.append(("proj", 1, qb, wi))
                for kt in range(4 * qb, 4 * qb + 4):
                    filler.append(("vtr", 1, kt))

            steps = [[(0, 0)], [(0, 1), (1, 0)], [(0, 2), (1, 1)],
                     [(0, 3), (1, 2)], [(1, 3)]]
            done_prev = []
            ctx_of = {}
            for si, streams in enumerate(steps):
                for (b, qb) in streams:
                    for hl in range(2):
                        ctx_of[(b, hl)] = ctxp.tile(
                            [65, 512], f32, tag=f"ctx{b}{hl}",
                            name=f"ctx{b}{hl}_{qb}")
                for kp in range(8):
                    for (b, qb) in streams:
                        emit_stream_kp(kp, b, qb, ctx_of)
                    if si == 0:
                        # ~3 filler items per kp group
                        while filler and len(filler) > (7 - kp) * 4:
                            kind, *args = filler.pop(0)
                            if kind == "proj":
                                emit_proj_chain(args[0], args[1], args[2])
                            else:
                                emit_vtrans(args[0], args[1])
                for (b, qb, snap) in done_prev:
                    emit_norm_wo(b, qb, snap)
                done_prev = [
                    (b, qb,
                     {(b, hl): ctx_of[(b, hl)] for hl in range(2)})
                    for (b, qb) in streams
                ]
            for (b, qb, snap) in done_prev:
                emit_norm_wo(b, qb, snap)

        if DEBUG_DUMPS:
            nc.sync.dma_start(dbg_qt[:], qt_sb[:])
            nc.sync.dma_start(dbg_kt[:], kt_sb[:])
            nc.sync.dma_start(dbg_v[:], v_sb[:])

    nc.compile()
    _PROGRAM_CACHE["nc"] = nc
    return nc


def run(inputs, trace=False, trace_kwargs=None):
    """Returns (full_output, BassKernelResults)."""
    from concourse.bass_utils import run_bass_kernel_spmd

    hidden_states = np.asarray(inputs["hidden_states"], dtype=np.float32)
    Wq = np.asarray(inputs["Wq"], dtype=np.float32)
    Wk = np.asarray(inputs["Wk"], dtype=np.float32)
    Wv = np.asarray(inputs["Wv"], dtype=np.float32)
    Wo = np.asarray(inputs["Wo"], dtype=np.float32)
    rel_emb = np.asarray(inputs["rel_emb"], dtype=np.float32)

    xt = np.ascontiguousarray(hidden_states.transpose(0, 2, 1))  # [B, D, S]
    xt_bf16 = np.ascontiguousarray(xt.reshape(B * 8, 128, S)).astype(BF16)

    nc = _build_program()
    in_maps = [
        _prep_core_inputs(c, hidden_states, Wq, Wk, Wv, Wo, rel_emb, xt_bf16)
        for c in range(N_CORES)
    ]
    res = run_bass_kernel_spmd(
        nc, in_maps, list(range(N_CORES)), trace=trace,
        **(trace_kwargs or {}),
    )
    out = np.zeros((B, S, D), dtype=np.float32)
    for c in range(N_CORES):
        out += res.results[c]["out"]
    return out, res


def kernel(**inputs):
    out, _ = run(inputs)
    return out


# revision 34
# speedup vs baseline: 1.0825x; 1.0266x over previous
"""T5-style relative-position-bias attention on 8 TRN2 NeuronCores.

Full-input contract: kernel(**inputs) takes the unsharded tensors and
returns the full [2, 2048, 1024] output.

Sharding: 16 heads / 8 cores = 2 heads per core, both batches on every
core (data stays identical; only weight shards differ). Each core
computes its partial output projection (its heads' contribution to the
full [B, S, D] output); the host sums the 8 partials.
"""

import math
import sys

sys.path.insert(0, "/opt/trn_rl_repo")

import numpy as np
import ml_dtypes

BF16 = ml_dtypes.bfloat16

B, S, D, H, HD = 2, 2048, 1024, 16, 64
N_CORES = 8
HEADS_PER_CORE = H // N_CORES  # 2
SCALING = HD ** (-0.5)
NUM_BUCKETS = 32
MAX_DISTANCE = 128

# q-block = 512 columns of the (transposed) score tile; k-tile = 128 rows.
QB = 512
KT = 128
N_QB = S // QB  # 4
N_KT = S // KT  # 16
# near-diagonal offsets m = kt - 4*qb for which bias varies inside the tile
NEAR_MS = list(range(-1, 5))  # -1..4


def _bucket_np(d):
    """Port of reference._relative_position_bucket (bidirectional), float32."""
    nb = NUM_BUCKETS // 2  # 16
    rb = (d > 0).astype(np.int32) * nb
    ad = np.abs(d)
    max_exact = nb // 2  # 8
    is_small = ad < max_exact
    rp = np.maximum(ad, 1).astype(np.float32)
    ril = max_exact + (
        np.log(rp / np.float32(max_exact))
        / np.float32(math.log(MAX_DISTANCE / max_exact))
        * np.float32(nb - max_exact)
    ).astype(np.int32)
    ril = np.minimum(ril, nb - 1)
    return rb + np.where(is_small, ad, ril)


def _near_bucket_tables():
    """Bucket index tile [128, 512] for each near offset m (head-independent)."""
    tables = {}
    p = np.arange(KT)[:, None]
    j = np.arange(QB)[None, :]
    for m in NEAR_MS:
        d = KT * m + p - j  # d = k - q
        tables[m] = _bucket_np(d)
    return tables


_NEAR_BUCKETS = _near_bucket_tables()


def _prep_core_inputs(c, hidden_states, Wq, Wk, Wv, Wo, rel_emb, xt_bf16):
    rows = slice(128 * c, 128 * (c + 1))
    wqt = np.ascontiguousarray(Wq[rows].T.reshape(8, 128, 128)).astype(BF16)
    wkt = np.ascontiguousarray(Wk[rows].T.reshape(8, 128, 128)).astype(BF16)
    wvt = np.ascontiguousarray(Wv[rows].T.reshape(8, 128, 128)).astype(BF16)
    wot = np.ascontiguousarray(Wo[:, rows].T).astype(BF16)  # [128, 1024]

    # E tiles: exp(bias) for near-diagonal tiles; [2 heads, 6 offsets, 128, 512]
    etab = np.empty((HEADS_PER_CORE * len(NEAR_MS), KT, QB), dtype=BF16)
    bfar = np.empty((4,), dtype=np.float32)
    for hl in range(HEADS_PER_CORE):
        h = HEADS_PER_CORE * c + hl
        for mi, m in enumerate(NEAR_MS):
            etab[hl * len(NEAR_MS) + mi] = np.exp(
                rel_emb[_NEAR_BUCKETS[m], h].astype(np.float32)
            ).astype(BF16)
        bfar[2 * hl + 0] = rel_emb[15, h]  # far negative (k << q)
        bfar[2 * hl + 1] = rel_emb[31, h]  # far positive (k >> q)
    bfar_t = np.tile(bfar[None, :], (128, 1)).astype(np.float32)
    bfarexp_t = np.exp(bfar_t).astype(np.float32)

    return {
        "xt": xt_bf16,
        "wqt": wqt,
        "wkt": wkt,
        "wvt": wvt,
        "wot": wot,
        "etab": etab,
        "bfar": bfar_t,
        "bfarexp": bfarexp_t,
    }


_PROGRAM_CACHE = {}
DEBUG_DUMPS = False
BUILD_LEVEL = 6  # 1=proj 2=+scores/exp 3=+ctx 4=+norm 5=wo-mm 6=full


def _build_program():
    if "nc" in _PROGRAM_CACHE:
        return _PROGRAM_CACHE["nc"]

    from contextlib import ExitStack

    import concourse.bass as bass
    import concourse.tile as tile
    from concourse import bacc, mybir
    from concourse.masks import make_identity

    f32 = mybir.dt.float32
    bf16 = mybir.dt.bfloat16
    Exp = mybir.ActivationFunctionType.Exp

    nc = bacc.Bacc("TRN2", target_bir_lowering=False, debug=False,
                   num_devices=N_CORES)

    xt_d = nc.dram_tensor("xt", [B * 8, 128, S], bf16, kind="ExternalInput").ap()
    wqt_d = nc.dram_tensor("wqt", [8, 128, 128], bf16, kind="ExternalInput").ap()
    wkt_d = nc.dram_tensor("wkt", [8, 128, 128], bf16, kind="ExternalInput").ap()
    wvt_d = nc.dram_tensor("wvt", [8, 128, 128], bf16, kind="ExternalInput").ap()
    wot_d = nc.dram_tensor("wot", [128, 1024], bf16, kind="ExternalInput").ap()
    etab_d = nc.dram_tensor("etab", [12, 128, 512], bf16, kind="ExternalInput").ap()
    bfar_d = nc.dram_tensor("bfar", [128, 4], f32, kind="ExternalInput").ap()
    bfarexp_d = nc.dram_tensor("bfarexp", [128, 4], f32,
                               kind="ExternalInput").ap()
    out_d = nc.dram_tensor("out", [B, S, D], f32, kind="ExternalOutput").ap()
    if DEBUG_DUMPS:
        dbg_qt = nc.dram_tensor("dbg_qt", [128, B * S], bf16,
                                kind="ExternalOutput").ap()
        dbg_kt = nc.dram_tensor("dbg_kt", [128, B * S], bf16,
                                kind="ExternalOutput").ap()
        dbg_v = nc.dram_tensor("dbg_v", [128, B * 16 * 130], bf16,
                               kind="ExternalOutput").ap()
        dbg_ut = nc.dram_tensor("dbg_ut", [128, 1024], bf16,
                                kind="ExternalOutput").ap()
        dbg_sct = nc.dram_tensor("dbg_sct", [128, 1024], f32,
                                 kind="ExternalOutput").ap()
        dbg_ctx = nc.dram_tensor("dbg_ctx", [2, 65, 512], f32,
                                 kind="ExternalOutput").ap()
        dbg_lct = nc.dram_tensor("dbg_lct", [2, 64, 512], bf16,
                                 kind="ExternalOutput").ap()
        dbg_rzb = nc.dram_tensor("dbg_rzb", [2, 64, 512], f32,
                                 kind="ExternalOutput").ap()

    VSLOT = 2 * 65  # [VA | 1 | VB | 1] per (b, kt)

    with tile.TileContext(nc) as tc, ExitStack() as ctx:
        const = ctx.enter_context(tc.tile_pool(name="const", bufs=1))

        xt_sb = const.tile([128, B * 8 * S], bf16, tag="xt")
        for i in range(B * 8):
            nc.sync.dma_start(xt_sb[:, S * i : S * (i + 1)], xt_d[i])
        wq_sb = const.tile([128, 8 * 128], bf16, tag="wq")
        wk_sb = const.tile([128, 8 * 128], bf16, tag="wk")
        wv_sb = const.tile([128, 8 * 128], bf16, tag="wv")
        for w_sb, w_d in ((wq_sb, wqt_d), (wk_sb, wkt_d), (wv_sb, wvt_d)):
            for i in range(8):
                nc.sync.dma_start(w_sb[:, 128 * i : 128 * (i + 1)], w_d[i])
        wot_sb = const.tile([128, 1024], bf16, tag="wot")
        nc.sync.dma_start(wot_sb[:], wot_d[:])
        etab_sb = const.tile([128, 12 * 512], bf16, tag="etab")
        for i in range(12):
            nc.sync.dma_start(etab_sb[:, 512 * i : 512 * (i + 1)], etab_d[i])
        bfar_sb = const.tile([128, 4], f32, tag="bfar")
        nc.sync.dma_start(bfar_sb[:], bfar_d[:])
        bfarexp_sb = const.tile([128, 4], f32, tag="bfarexp")
        nc.sync.dma_start(bfarexp_sb[:], bfarexp_d[:])
        ident = const.tile([128, 128], bf16, tag="ident")
        make_identity(nc, ident[:])

        qt_sb = const.tile([128, B * S], bf16, tag="qt")
        vt_sb = const.tile([128, B * S], bf16, tag="vtt")
        kt_sb = const.tile([128, B * S], bf16, tag="kt")
        v_sb = const.tile([128, B * N_KT * VSLOT], bf16, tag="v")
        for b in range(B):
            for kt in range(N_KT):
                base = (b * N_KT + kt) * VSLOT
                nc.gpsimd.memset(v_sb[:, base + 64 : base + 65], 1.0)
                nc.gpsimd.memset(v_sb[:, base + 129 : base + 130], 1.0)

        # ---- Phase B: projections (transposed Q/K/V, weight stationary) ----
        with tc.tile_pool(name="projp", bufs=3, space="PSUM") as projp, \
             tc.tile_pool(name="vtrp", bufs=2, space="PSUM") as vtrp:

            def emit_proj(b):
                for qb in range(N_QB):
                    for wi, (w_sb, dst) in enumerate(((wq_sb, qt_sb),
                                                     (wk_sb, kt_sb),
                                                     (wv_sb, vt_sb))):
                        ps = projp.tile([128, 512], f32, tag="proj",
                                        name=f"pj_{b}_{qb}_{wi}")
                        for dt in range(8):
                            nc.tensor.matmul(
                                ps[:],
                                lhsT=w_sb[:, 128 * dt : 128 * (dt + 1)],
                                rhs=xt_sb[:, (b * 8 + dt) * S + qb * QB :
                                          (b * 8 + dt) * S + qb * QB + QB],
                                start=(dt == 0), stop=(dt == 7),
                            )
                        nc.vector.tensor_copy(
                            dst[:, b * S + qb * QB : b * S + qb * QB + QB],
                            ps[:]
                        )

            def emit_vtrans(b):
                for kt in range(N_KT):
                    tr = vtrp.tile([128, 128], bf16, tag="vtr",
                                   name=f"vtr_{b}_{kt}")
                    nc.tensor.transpose(
                        tr[:],
                        vt_sb[:, b * S + kt * KT : b * S + kt * KT + KT],
                        ident[:],
                    )
                    base = (b * N_KT + kt) * VSLOT
                    nc.vector.tensor_copy(v_sb[:, base : base + 64],
                                          tr[:, 0:64])
                    nc.vector.tensor_copy(v_sb[:, base + 65 : base + 129],
                                          tr[:, 64:128])

            emit_proj(0)
            emit_proj(1)
            emit_vtrans(0)
            emit_vtrans(1)

        # ---- Phase C: attention, batches interleaved; norm/wo pipelined ----
        def cls(m):
            if m <= -2:
                return 0  # far negative
            if m >= 5:
                return 1  # far positive
            return 2  # near

        with tc.tile_pool(name="scp", bufs=2, space="PSUM") as scp, \
             tc.tile_pool(name="ctxp", bufs=1, space="PSUM") as ctxp, \
             tc.tile_pool(name="utp", bufs=8) as utp, \
             tc.tile_pool(name="ostg", bufs=4) as ostg, \
             tc.tile_pool(name="nrm", bufs=1) as nrm:

            def emit_stream_kp(kp, b, qb, ctxs):
                m0 = 2 * kp - 4 * qb
                m1 = m0 + 1
                c0, c1 = cls(m0), cls(m1)
                for hl in range(2):
                    sct = scp.tile([128, 1024], f32, tag="sc",
                                   name=f"sc_{qb}_{kp}_{b}_{hl}")
                    for half in range(2):
                        kt = 2 * kp + half
                        nc.tensor.matmul(
                            sct[:, 512 * half : 512 * (half + 1)],
                            lhsT=kt_sb[64 * hl : 64 * (hl + 1),
                                       b * S + kt * KT : b * S + kt * KT + KT],
                            rhs=qt_sb[64 * hl : 64 * (hl + 1),
                                      b * S + qb * QB : b * S + qb * QB + QB],
                            start=True, stop=True,
                        )
                    ut = utp.tile([128, 1024], bf16, tag="ut",
                                  name=f"ut_{qb}_{kp}_{b}_{hl}")
                    srcs = [(ut, 0), (ut, 512)]
                    if c0 == c1 and c0 != 2:
                        nc.scalar.activation(
                            ut[:], sct[:], Exp,
                            bias=bfar_sb[:, 2 * hl + c0 : 2 * hl + c0 + 1],
                            scale=SCALING,
                        )
                    else:
                        nc.scalar.activation(
                            ut[:], sct[:], Exp, bias=0.0, scale=SCALING
                        )
                        ut2 = utp.tile([128, 1024], bf16, tag="ut2",
                                       name=f"ut2_{qb}_{kp}_{b}_{hl}")
                        if c0 == c1 == 2:
                            ei = (hl * 6 + (m0 + 1)) * 512
                            nc.vector.tensor_mul(
                                ut2[:], ut[:], etab_sb[:, ei : ei + 1024]
                            )
                            srcs = [(ut2, 0), (ut2, 512)]
                        else:
                            for half, (m, cc) in enumerate(((m0, c0),
                                                           (m1, c1))):
                                usl = ut[:, 512 * half : 512 * (half + 1)]
                                osl = ut2[:, 512 * half : 512 * (half + 1)]
                                if cc == 2:
                                    ei = (hl * 6 + (m + 1)) * 512
                                    nc.vector.tensor_mul(
                                        osl, usl, etab_sb[:, ei : ei + 512]
                                    )
                                else:
                                    col = 2 * hl + cc
                                    nc.vector.tensor_scalar_mul(
                                        osl, usl, bfarexp_sb[:, col : col + 1]
                                    )
                                srcs[half] = (ut2, 512 * half)
                    for half in range(2):
                        kt = 2 * kp + half
                        base = (b * N_KT + kt) * VSLOT + 65 * hl
                        stile, soff = srcs[half]
                        nc.tensor.matmul(
                            ctxs[(b, hl)][:],
                            lhsT=v_sb[:, base : base + 65],
                            rhs=stile[:, soff : soff + 512],
                            start=(kt == 0), stop=(kt == N_KT - 1),
                        )

            def emit_norm_wo(b, qb, ctxs):
                lct = nrm.tile([128, 512], bf16, tag=f"lct{b}",
                               name=f"lct{b}_{qb}", bufs=2)
                for hl in range(2):
                    rz = nrm.tile([128, 512], f32, tag=f"rz{b}{hl}",
                                  name=f"rz{b}{hl}_{qb}")
                    nc.vector.tensor_copy(rz[0:1, :], ctxs[(b, hl)][64:65, :])
                    rzf = nrm.tile([128, 512], f32, tag=f"rzf{b}{hl}",
                                   name=f"rzf{b}{hl}_{qb}")
                    nc.vector.reciprocal_approx_fast(
                        out=rzf[0:1, :], in_=rz[0:1, :]
                    )
                    rzb = nrm.tile([64, 512], f32, tag=f"rzb{b}{hl}",
                                   name=f"rzb{b}{hl}_{qb}")
                    nc.gpsimd.partition_broadcast(
                        rzb[:], rzf[0:1, :], channels=64
                    )
                    nc.vector.tensor_mul(
                        lct[64 * hl : 64 * (hl + 1), :],
                        ctxs[(b, hl)][0:64, :], rzb[:])
                for st in range(4):
                    wo_ps = scp.tile([128, 1024], f32, tag="sc",
                                     name=f"wo_{qb}_{b}_{st}")
                    for nh in range(2):
                        nc.tensor.matmul(
                            wo_ps[:, nh * 512 : (nh + 1) * 512],
                            lhsT=lct[:, st * 128 : (st + 1) * 128],
                            rhs=wot_sb[:, nh * 512 : (nh + 1) * 512],
                            start=True, stop=True,
                        )
                    ot = ostg.tile([128, 1024], f32, tag="ot",
                                   name=f"ot_{qb}_{b}_{st}")
                    nc.vector.tensor_copy(ot[:], wo_ps[:])
                    srow = qb * QB + st * 128
                    nc.gpsimd.dma_start(out_d[b, srow : srow + 128, :], ot[:])

            done_prev = None
            for qb in range(N_QB):
                ctxs = {}
                for b in range(B):
                    for hl in range(2):
                        ctxs[(b, hl)] = ctxp.tile(
                            [65, 512], f32, tag=f"ctx{b}{hl}",
                            name=f"ctx{b}{hl}_{qb}")
                for kp in range(8):
                    for b in range(B):
                        emit_stream_kp(kp, b, qb, ctxs)
                if done_prev is not None:
                    pq, pc = done_prev
                    for b in range(B):
                        emit_norm_wo(b, pq, pc)
                done_prev = (qb, ctxs)
            pq, pc = done_prev
            for b in range(B):
                emit_norm_wo(b, pq, pc)

        if DEBUG_DUMPS:
            nc.sync.dma_start(dbg_qt[:], qt_sb[:])
            nc.sync.dma_start(dbg_kt[:], kt_sb[:])
            nc.sync.dma_start(dbg_v[:], v_sb[:])

    nc.compile()
    _PROGRAM_CACHE["nc"] = nc
    return nc


def run(inputs, trace=False, trace_kwargs=None):
    """Returns (full_output, BassKernelResults)."""
    from concourse.bass_utils import run_bass_kernel_spmd

    hidden_states = np.asarray(inputs["hidden_states"], dtype=np.float32)
    Wq = np.asarray(inputs["Wq"], dtype=np.float32)
    Wk = np.asarray(inputs["Wk"], dtype=np.float32)
    Wv = np.asarray(inputs["Wv"], dtype=np.float32)
    Wo = np.asarray(inputs["Wo"], dtype=np.float32)
    rel_emb = np.asarray(inputs["rel_emb"], dtype=np.float32)

    xt = np.ascontiguousarray(hidden_states.transpose(0, 2, 1))  # [B, D, S]
    xt_bf16 = np.ascontiguousarray(xt.reshape(B * 8, 128, S)).astype(BF16)

    nc = _build_program()
    in_maps = [
        _prep_core_inputs(c, hidden_states, Wq, Wk, Wv, Wo, rel_emb, xt_bf16)
        for c in range(N_CORES)
    ]
    res = run_bass_kernel_spmd(
        nc, in_maps, list(range(N_CORES)), trace=trace,
        **(trace_kwargs or {}),
    )
    out = np.zeros((B, S, D), dtype=np.float32)
    for c in range(N_CORES):
        out += res.results[c]["out"]
    return out, res


def kernel(**inputs):
    out, _ = run(inputs)
    return out


# revision 37
# speedup vs baseline: 1.2895x; 1.1912x over previous
"""T5-style relative-position-bias attention on 8 TRN2 NeuronCores.

Full-input contract: kernel(**inputs) takes the unsharded tensors and
returns the full [2, 2048, 1024] output.

Sharding: 16 heads / 8 cores = 2 heads per core, both batches on every
core (data stays identical; only weight shards differ). Each core
computes its partial output projection (its heads' contribution to the
full [B, S, D] output); the host sums the 8 partials.
"""

import math
import sys

sys.path.insert(0, "/opt/trn_rl_repo")

import numpy as np
import ml_dtypes

BF16 = ml_dtypes.bfloat16

B, S, D, H, HD = 2, 2048, 1024, 16, 64
N_CORES = 8
HEADS_PER_CORE = H // N_CORES  # 2
SCALING = HD ** (-0.5)
NUM_BUCKETS = 32
MAX_DISTANCE = 128

# q-block = 512 columns of the (transposed) score tile; k-tile = 128 rows.
QB = 512
KT = 128
N_QB = S // QB  # 4
N_KT = S // KT  # 16
# near-diagonal offsets m = kt - 4*qb for which bias varies inside the tile
NEAR_MS = list(range(-1, 5))  # -1..4


def _bucket_np(d):
    """Port of reference._relative_position_bucket (bidirectional), float32."""
    nb = NUM_BUCKETS // 2  # 16
    rb = (d > 0).astype(np.int32) * nb
    ad = np.abs(d)
    max_exact = nb // 2  # 8
    is_small = ad < max_exact
    rp = np.maximum(ad, 1).astype(np.float32)
    ril = max_exact + (
        np.log(rp / np.float32(max_exact))
        / np.float32(math.log(MAX_DISTANCE / max_exact))
        * np.float32(nb - max_exact)
    ).astype(np.int32)
    ril = np.minimum(ril, nb - 1)
    return rb + np.where(is_small, ad, ril)


def _near_bucket_tables():
    """Bucket index tile [128, 512] for each near offset m (head-independent)."""
    tables = {}
    p = np.arange(KT)[:, None]
    j = np.arange(QB)[None, :]
    for m in NEAR_MS:
        d = KT * m + p - j  # d = k - q
        tables[m] = _bucket_np(d)
    return tables


_NEAR_BUCKETS = _near_bucket_tables()


def _prep_core_inputs(c, hidden_states, Wq, Wk, Wv, Wo, rel_emb, xt_bf16):
    rows = slice(128 * c, 128 * (c + 1))
    wqt = np.ascontiguousarray(Wq[rows].T.reshape(8, 128, 128)).astype(BF16)
    wkt = np.ascontiguousarray(Wk[rows].T.reshape(8, 128, 128)).astype(BF16)
    wvt = np.ascontiguousarray(Wv[rows].T.reshape(8, 128, 128)).astype(BF16)
    wot = np.ascontiguousarray(Wo[:, rows].T).astype(BF16)  # [128, 1024]

    # E tiles: exp(bias) for near-diagonal tiles; [2 heads, 6 offsets, 128, 512]
    etab = np.empty((HEADS_PER_CORE * len(NEAR_MS), KT, QB), dtype=BF16)
    bfar = np.empty((4,), dtype=np.float32)
    for hl in range(HEADS_PER_CORE):
        h = HEADS_PER_CORE * c + hl
        for mi, m in enumerate(NEAR_MS):
            etab[hl * len(NEAR_MS) + mi] = np.exp(
                rel_emb[_NEAR_BUCKETS[m], h].astype(np.float32)
            ).astype(BF16)
        bfar[2 * hl + 0] = rel_emb[15, h]  # far negative (k << q)
        bfar[2 * hl + 1] = rel_emb[31, h]  # far positive (k >> q)
    bfar_t = np.tile(bfar[None, :], (128, 1)).astype(np.float32)
    bfarexp_t = np.exp(bfar_t).astype(np.float32)

    return {
        "xt": xt_bf16,
        "wqt": wqt,
        "wkt": wkt,
        "wvt": wvt,
        "wot": wot,
        "etab": etab,
        "bfar": bfar_t,
        "bfarexp": bfarexp_t,
    }


_PROGRAM_CACHE = {}
DEBUG_DUMPS = False
BUILD_LEVEL = 6  # 1=proj 2=+scores/exp 3=+ctx 4=+norm 5=wo-mm 6=full


def _build_program():
    if "nc" in _PROGRAM_CACHE:
        return _PROGRAM_CACHE["nc"]

    from contextlib import ExitStack

    import concourse.bass as bass
    import concourse.tile as tile
    from concourse import bacc, mybir
    from concourse.masks import make_identity

    f32 = mybir.dt.float32
    bf16 = mybir.dt.bfloat16
    Exp = mybir.ActivationFunctionType.Exp

    nc = bacc.Bacc("TRN2", target_bir_lowering=False, debug=False,
                   num_devices=N_CORES)

    xt_d = nc.dram_tensor("xt", [B * 8, 128, S], bf16, kind="ExternalInput").ap()
    wqt_d = nc.dram_tensor("wqt", [8, 128, 128], bf16, kind="ExternalInput").ap()
    wkt_d = nc.dram_tensor("wkt", [8, 128, 128], bf16, kind="ExternalInput").ap()
    wvt_d = nc.dram_tensor("wvt", [8, 128, 128], bf16, kind="ExternalInput").ap()
    wot_d = nc.dram_tensor("wot", [128, 1024], bf16, kind="ExternalInput").ap()
    etab_d = nc.dram_tensor("etab", [12, 128, 512], bf16, kind="ExternalInput").ap()
    bfar_d = nc.dram_tensor("bfar", [128, 4], f32, kind="ExternalInput").ap()
    bfarexp_d = nc.dram_tensor("bfarexp", [128, 4], f32,
                               kind="ExternalInput").ap()
    out_d = nc.dram_tensor("out", [B, S, D], f32, kind="ExternalOutput").ap()
    if DEBUG_DUMPS:
        dbg_qt = nc.dram_tensor("dbg_qt", [128, B * S], bf16,
                                kind="ExternalOutput").ap()
        dbg_kt = nc.dram_tensor("dbg_kt", [128, B * S], bf16,
                                kind="ExternalOutput").ap()
        dbg_v = nc.dram_tensor("dbg_v", [128, B * 16 * 130], bf16,
                               kind="ExternalOutput").ap()
        dbg_ut = nc.dram_tensor("dbg_ut", [128, 1024], bf16,
                                kind="ExternalOutput").ap()
        dbg_sct = nc.dram_tensor("dbg_sct", [128, 1024], f32,
                                 kind="ExternalOutput").ap()
        dbg_ctx = nc.dram_tensor("dbg_ctx", [2, 65, 512], f32,
                                 kind="ExternalOutput").ap()
        dbg_lct = nc.dram_tensor("dbg_lct", [2, 64, 512], bf16,
                                 kind="ExternalOutput").ap()
        dbg_rzb = nc.dram_tensor("dbg_rzb", [2, 64, 512], f32,
                                 kind="ExternalOutput").ap()

    VSLOT = 2 * 65  # [VA | 1 | VB | 1] per (b, kt)

    with tile.TileContext(nc) as tc, ExitStack() as ctx:
        const = ctx.enter_context(tc.tile_pool(name="const", bufs=1))

        xt_sb = const.tile([128, B * 8 * S], bf16, tag="xt")
        wq_sb = const.tile([128, 8 * 128], bf16, tag="wq")
        wk_sb = const.tile([128, 8 * 128], bf16, tag="wk")
        wv_sb = const.tile([128, 8 * 128], bf16, tag="wv")
        # small tensors first so the projection chains can start while the
        # bulk of xt is still in flight
        for w_sb, w_d in ((wq_sb, wqt_d), (wk_sb, wkt_d), (wv_sb, wvt_d)):
            for i in range(8):
                nc.sync.dma_start(w_sb[:, 128 * i : 128 * (i + 1)], w_d[i])
        wot_sb = const.tile([128, 1024], bf16, tag="wot")
        nc.sync.dma_start(wot_sb[:], wot_d[:])
        bfar_sb = const.tile([128, 4], f32, tag="bfar")
        nc.sync.dma_start(bfar_sb[:], bfar_d[:])
        bfarexp_sb = const.tile([128, 4], f32, tag="bfarexp")
        nc.sync.dma_start(bfarexp_sb[:], bfarexp_d[:])
        etab_sb = const.tile([128, 12 * 512], bf16, tag="etab")
        for i in range(12):
            nc.sync.dma_start(etab_sb[:, 512 * i : 512 * (i + 1)], etab_d[i])
        for i in range(B * 8):
            nc.sync.dma_start(xt_sb[:, S * i : S * (i + 1)], xt_d[i])
        ident = const.tile([128, 128], bf16, tag="ident")
        make_identity(nc, ident[:])

        qt_sb = const.tile([128, B * S], bf16, tag="qt")
        vt_sb = const.tile([128, B * S], bf16, tag="vtt")
        kt_sb = const.tile([128, B * S], bf16, tag="kt")
        v_sb = const.tile([128, B * N_KT * VSLOT], bf16, tag="v")
        for b in range(B):
            for kt in range(N_KT):
                base = (b * N_KT + kt) * VSLOT
                nc.gpsimd.memset(v_sb[:, base + 64 : base + 65], 1.0)
                nc.gpsimd.memset(v_sb[:, base + 129 : base + 130], 1.0)

        # ---- Phases B+C staggered ----
        # PE and ACT total work are nearly equal, but projections are
        # PE-only while attention is ACT-paced. Batch 0's projections run
        # first; batch 1's projections fill the PE during batch 0's first
        # attention block; then the two batches' q-blocks run offset by one
        # step. PSUM: sct 4 banks + ctx(b0) 2 + proj 2, and the proj pool is
        # swapped for ctx(b1)'s banks after step 0.
        def cls(m):
            if m <= -2:
                return 0  # far negative
            if m >= 5:
                return 1  # far positive
            return 2  # near

        with tc.tile_pool(name="scp", bufs=2, space="PSUM") as scp, \
             tc.tile_pool(name="ctxp0", bufs=1, space="PSUM") as ctxp0, \
             tc.tile_pool(name="utp", bufs=8) as utp, \
             tc.tile_pool(name="ostg", bufs=4) as ostg, \
             tc.tile_pool(name="nrm", bufs=1) as nrm:

            projp = tc.alloc_tile_pool(name="projp", bufs=2, space="PSUM")

            def emit_qk_chain(b, qb, wi):
                w_sb, dst = ((wq_sb, qt_sb), (wk_sb, kt_sb))[wi]
                ps = projp.tile([128, 512], f32, tag="proj",
                                name=f"pj_{b}_{qb}_{wi}")
                for dt in range(8):
                    nc.tensor.matmul(
                        ps[:],
                        lhsT=w_sb[:, 128 * dt : 128 * (dt + 1)],
                        rhs=xt_sb[:, (b * 8 + dt) * S + qb * QB :
                                  (b * 8 + dt) * S + qb * QB + QB],
                        start=(dt == 0), stop=(dt == 7),
                    )
                nc.vector.tensor_copy(
                    dst[:, b * S + qb * QB : b * S + qb * QB + QB], ps[:]
                )

            def emit_v_chain(b, st):
                ps = projp.tile([128, 128], f32, tag="proj",
                                name=f"vp_{b}_{st}")
                for dt in range(8):
                    nc.tensor.matmul(
                        ps[:],
                        lhsT=xt_sb[:, (b * 8 + dt) * S + st * KT :
                                   (b * 8 + dt) * S + st * KT + KT],
                        rhs=wv_sb[:, 128 * dt : 128 * (dt + 1)],
                        start=(dt == 0), stop=(dt == 7),
                    )
                base = (b * N_KT + st) * VSLOT
                nc.vector.tensor_copy(v_sb[:, base : base + 64], ps[:, 0:64])
                nc.vector.tensor_copy(v_sb[:, base + 65 : base + 129],
                                      ps[:, 64:128])

            def emit_stream_kp(kp, b, qb, ctxs):
                m0 = 2 * kp - 4 * qb
                m1 = m0 + 1
                c0, c1 = cls(m0), cls(m1)
                for hl in range(2):
                    sct = scp.tile([128, 1024], f32, tag="sc",
                                   name=f"sc_{qb}_{kp}_{b}_{hl}")
                    for half in range(2):
                        kt = 2 * kp + half
                        nc.tensor.matmul(
                            sct[:, 512 * half : 512 * (half + 1)],
                            lhsT=kt_sb[64 * hl : 64 * (hl + 1),
                                       b * S + kt * KT : b * S + kt * KT + KT],
                            rhs=qt_sb[64 * hl : 64 * (hl + 1),
                                      b * S + qb * QB : b * S + qb * QB + QB],
                            start=True, stop=True,
                        )
                    ut = utp.tile([128, 1024], bf16, tag="ut",
                                  name=f"ut_{qb}_{kp}_{b}_{hl}")
                    srcs = [(ut, 0), (ut, 512)]
                    if c0 == c1 and c0 != 2:
                        nc.scalar.activation(
                            ut[:], sct[:], Exp,
                            bias=bfar_sb[:, 2 * hl + c0 : 2 * hl + c0 + 1],
                            scale=SCALING,
                        )
                    else:
                        nc.scalar.activation(
                            ut[:], sct[:], Exp, bias=0.0, scale=SCALING
                        )
                        ut2 = utp.tile([128, 1024], bf16, tag="ut2",
                                       name=f"ut2_{qb}_{kp}_{b}_{hl}")
                        if c0 == c1 == 2:
                            ei = (hl * 6 + (m0 + 1)) * 512
                            nc.vector.tensor_mul(
                                ut2[:], ut[:], etab_sb[:, ei : ei + 1024]
                            )
                            srcs = [(ut2, 0), (ut2, 512)]
                        else:
                            for half, (m, cc) in enumerate(((m0, c0),
                                                           (m1, c1))):
                                usl = ut[:, 512 * half : 512 * (half + 1)]
                                osl = ut2[:, 512 * half : 512 * (half + 1)]
                                if cc == 2:
                                    ei = (hl * 6 + (m + 1)) * 512
                                    nc.vector.tensor_mul(
                                        osl, usl, etab_sb[:, ei : ei + 512]
                                    )
                                else:
                                    col = 2 * hl + cc
                                    nc.vector.tensor_scalar_mul(
                                        osl, usl, bfarexp_sb[:, col : col + 1]
                                    )
                                srcs[half] = (ut2, 512 * half)
                    for half in range(2):
                        kt = 2 * kp + half
                        base = (b * N_KT + kt) * VSLOT + 65 * hl
                        stile, soff = srcs[half]
                        nc.tensor.matmul(
                            ctxs[(b, hl)][:],
                            lhsT=v_sb[:, base : base + 65],
                            rhs=stile[:, soff : soff + 512],
                            start=(kt == 0), stop=(kt == N_KT - 1),
                        )

            def emit_norm_wo(b, qb, ctxs):
                lct = nrm.tile([128, 512], bf16, tag=f"lct{b}",
                               name=f"lct{b}_{qb}", bufs=2)
                for hl in range(2):
                    rz = nrm.tile([128, 512], f32, tag=f"rz{b}{hl}",
                                  name=f"rz{b}{hl}_{qb}")
                    nc.vector.tensor_copy(rz[0:1, :], ctxs[(b, hl)][64:65, :])
                    rzf = nrm.tile([128, 512], f32, tag=f"rzf{b}{hl}",
                                   name=f"rzf{b}{hl}_{qb}")
                    nc.vector.reciprocal_approx_fast(
                        out=rzf[0:1, :], in_=rz[0:1, :]
                    )
                    rzb = nrm.tile([64, 512], f32, tag=f"rzb{b}{hl}",
                                   name=f"rzb{b}{hl}_{qb}")
                    nc.gpsimd.partition_broadcast(
                        rzb[:], rzf[0:1, :], channels=64
                    )
                    nc.vector.tensor_mul(
                        lct[64 * hl : 64 * (hl + 1), :],
                        ctxs[(b, hl)][0:64, :], rzb[:])
                for st in range(4):
                    wo_ps = scp.tile([128, 1024], f32, tag="sc",
                                     name=f"wo_{qb}_{b}_{st}")
                    for nh in range(2):
                        nc.tensor.matmul(
                            wo_ps[:, nh * 512 : (nh + 1) * 512],
                            lhsT=lct[:, st * 128 : (st + 1) * 128],
                            rhs=wot_sb[:, nh * 512 : (nh + 1) * 512],
                            start=True, stop=True,
                        )
                    ot = ostg.tile([128, 1024], f32, tag="ot",
                                   name=f"ot_{qb}_{b}_{st}")
                    nc.vector.tensor_copy(ot[:], wo_ps[:])
                    srow = qb * QB + st * 128
                    nc.sync.dma_start(out_d[b, srow : srow + 128, :], ot[:])

            # batch-0 projections
            for qb in range(N_QB):
                emit_qk_chain(0, qb, 0)
                emit_qk_chain(0, qb, 1)
                for st in (4 * qb, 4 * qb + 1, 4 * qb + 2, 4 * qb + 3):
                    emit_v_chain(0, st)

            # step 0: batch-0 qb0 attention + batch-1 projections as filler
            filler = []
            for qb in range(N_QB):
                filler.append(("qk", qb, 0))
                filler.append(("qk", qb, 1))
                for st in (4 * qb, 4 * qb + 1, 4 * qb + 2, 4 * qb + 3):
                    filler.append(("v", st))
            ctx_b0 = {}
            ctx_pool_of = {0: ctxp0}
            for hl in range(2):
                ctx_b0[(0, hl)] = ctxp0.tile([65, 512], f32, tag=f"c0{hl}",
                                             name=f"ctx0{hl}_0")
            for kp in range(8):
                emit_stream_kp(kp, 0, 0, ctx_b0)
                while filler and len(filler) > (7 - kp) * 3:
                    kind, *args = filler.pop(0)
                    if kind == "qk":
                        emit_qk_chain(1, args[0], args[1])
                    else:
                        emit_v_chain(1, args[0])

            projp.release()
            ctxp1 = tc.alloc_tile_pool(name="ctxp1", bufs=1, space="PSUM")

            # steps 1..4: offset schedule + pipelined norm/wo
            def alloc_ctx(b, qb):
                pool = ctxp0 if b == 0 else ctxp1
                d = {}
                for hl in range(2):
                    d[(b, hl)] = pool.tile([65, 512], f32, tag=f"c{b}{hl}",
                                           name=f"ctx{b}{hl}_{qb}")
                return d

            done_prev = [(0, 0, ctx_b0)]
            steps = [[(0, 1), (1, 0)], [(0, 2), (1, 1)], [(0, 3), (1, 2)],
                     [(1, 3)]]
            for streams in steps:
                ctxs_list = [(b, qb, alloc_ctx(b, qb)) for (b, qb) in streams]
                for kp in range(8):
                    for (b, qb, cd) in ctxs_list:
                        emit_stream_kp(kp, b, qb, cd)
                for (b, qb, cd) in done_prev:
                    emit_norm_wo(b, qb, cd)
                done_prev = ctxs_list
            for (b, qb, cd) in done_prev:
                emit_norm_wo(b, qb, cd)
            ctxp1.release()

        if DEBUG_DUMPS:
            nc.sync.dma_start(dbg_qt[:], qt_sb[:])
            nc.sync.dma_start(dbg_kt[:], kt_sb[:])
            nc.sync.dma_start(dbg_v[:], v_sb[:])

    nc.compile()
    _PROGRAM_CACHE["nc"] = nc
    return nc


def run(inputs, trace=False, trace_kwargs=None):
    """Returns (full_output, BassKernelResults)."""
    from concourse.bass_utils import run_bass_kernel_spmd

    hidden_states = np.asarray(inputs["hidden_states"], dtype=np.float32)
    Wq = np.asarray(inputs["Wq"], dtype=np.float32)
    Wk = np.asarray(inputs["Wk"], dtype=np.float32)
    Wv = np.asarray(inputs["Wv"], dtype=np.float32)
    Wo = np.asarray(inputs["Wo"], dtype=np.float32)
    rel_emb = np.asarray(inputs["rel_emb"], dtype=np.float32)

    xt = np.ascontiguousarray(hidden_states.transpose(0, 2, 1))  # [B, D, S]
    xt_bf16 = np.ascontiguousarray(xt.reshape(B * 8, 128, S)).astype(BF16)

    nc = _build_program()
    in_maps = [
        _prep_core_inputs(c, hidden_states, Wq, Wk, Wv, Wo, rel_emb, xt_bf16)
        for c in range(N_CORES)
    ]
    res = run_bass_kernel_spmd(
        nc, in_maps, list(range(N_CORES)), trace=trace,
        **(trace_kwargs or {}),
    )
    out = np.zeros((B, S, D), dtype=np.float32)
    for c in range(N_CORES):
        out += res.results[c]["out"]
    return out, res


def kernel(**inputs):
    out, _ = run(inputs)
    return out


# revision 38
# speedup vs baseline: 1.3817x; 1.0715x over previous
"""T5-style relative-position-bias attention on 8 TRN2 NeuronCores.

Full-input contract: kernel(**inputs) takes the unsharded tensors and
returns the full [2, 2048, 1024] output.

Sharding: 16 heads / 8 cores = 2 heads per core, both batches on every
core (data stays identical; only weight shards differ). Each core
computes its partial output projection (its heads' contribution to the
full [B, S, D] output); the host sums the 8 partials.
"""

import math
import sys

sys.path.insert(0, "/opt/trn_rl_repo")

import numpy as np
import ml_dtypes

BF16 = ml_dtypes.bfloat16

B, S, D, H, HD = 2, 2048, 1024, 16, 64
N_CORES = 8
HEADS_PER_CORE = H // N_CORES  # 2
SCALING = HD ** (-0.5)
NUM_BUCKETS = 32
MAX_DISTANCE = 128

# q-block = 512 columns of the (transposed) score tile; k-tile = 128 rows.
QB = 512
KT = 128
N_QB = S // QB  # 4
N_KT = S // KT  # 16
# near-diagonal offsets m = kt - 4*qb for which bias varies inside the tile
NEAR_MS = list(range(-1, 5))  # -1..4


def _bucket_np(d):
    """Port of reference._relative_position_bucket (bidirectional), float32."""
    nb = NUM_BUCKETS // 2  # 16
    rb = (d > 0).astype(np.int32) * nb
    ad = np.abs(d)
    max_exact = nb // 2  # 8
    is_small = ad < max_exact
    rp = np.maximum(ad, 1).astype(np.float32)
    ril = max_exact + (
        np.log(rp / np.float32(max_exact))
        / np.float32(math.log(MAX_DISTANCE / max_exact))
        * np.float32(nb - max_exact)
    ).astype(np.int32)
    ril = np.minimum(ril, nb - 1)
    return rb + np.where(is_small, ad, ril)


def _near_bucket_tables():
    """Bucket index tile [128, 512] for each near offset m (head-independent)."""
    tables = {}
    p = np.arange(KT)[:, None]
    j = np.arange(QB)[None, :]
    for m in NEAR_MS:
        d = KT * m + p - j  # d = k - q
        tables[m] = _bucket_np(d)
    return tables


_NEAR_BUCKETS = _near_bucket_tables()


def _prep_core_inputs(c, hidden_states, Wq, Wk, Wv, Wo, rel_emb, xt_bf16):
    rows = slice(128 * c, 128 * (c + 1))
    wqt = np.ascontiguousarray(Wq[rows].T.reshape(8, 128, 128)).astype(BF16)
    wkt = np.ascontiguousarray(Wk[rows].T.reshape(8, 128, 128)).astype(BF16)
    wvt = np.ascontiguousarray(Wv[rows].T.reshape(8, 128, 128)).astype(BF16)
    wot = np.ascontiguousarray(Wo[:, rows].T).astype(BF16)  # [128, 1024]

    # E tiles: exp(bias) for near-diagonal tiles; [2 heads, 6 offsets, 128, 512]
    etab = np.empty((HEADS_PER_CORE * len(NEAR_MS), KT, QB), dtype=BF16)
    bfar = np.empty((4,), dtype=np.float32)
    for hl in range(HEADS_PER_CORE):
        h = HEADS_PER_CORE * c + hl
        for mi, m in enumerate(NEAR_MS):
            etab[hl * len(NEAR_MS) + mi] = np.exp(
                rel_emb[_NEAR_BUCKETS[m], h].astype(np.float32)
            ).astype(BF16)
        bfar[2 * hl + 0] = rel_emb[15, h]  # far negative (k << q)
        bfar[2 * hl + 1] = rel_emb[31, h]  # far positive (k >> q)
    bfar_t = np.tile(bfar[None, :], (128, 1)).astype(np.float32)
    bfarexp_t = np.exp(bfar_t).astype(np.float32)

    return {
        "xt": xt_bf16,
        "wqt": wqt,
        "wkt": wkt,
        "wvt": wvt,
        "wot": wot,
        "etab": etab,
        "bfar": bfar_t,
        "bfarexp": bfarexp_t,
    }


_PROGRAM_CACHE = {}
DEBUG_DUMPS = False
BUILD_LEVEL = 6  # 1=proj 2=+scores/exp 3=+ctx 4=+norm 5=wo-mm 6=full


def _build_program():
    if "nc" in _PROGRAM_CACHE:
        return _PROGRAM_CACHE["nc"]

    from contextlib import ExitStack

    import concourse.bass as bass
    import concourse.tile as tile
    from concourse import bacc, mybir
    from concourse.masks import make_identity

    f32 = mybir.dt.float32
    bf16 = mybir.dt.bfloat16
    Exp = mybir.ActivationFunctionType.Exp

    nc = bacc.Bacc("TRN2", target_bir_lowering=False, debug=False,
                   num_devices=N_CORES)

    xt_d = nc.dram_tensor("xt", [B * 8, 128, S], bf16, kind="ExternalInput").ap()
    wqt_d = nc.dram_tensor("wqt", [8, 128, 128], bf16, kind="ExternalInput").ap()
    wkt_d = nc.dram_tensor("wkt", [8, 128, 128], bf16, kind="ExternalInput").ap()
    wvt_d = nc.dram_tensor("wvt", [8, 128, 128], bf16, kind="ExternalInput").ap()
    wot_d = nc.dram_tensor("wot", [128, 1024], bf16, kind="ExternalInput").ap()
    etab_d = nc.dram_tensor("etab", [12, 128, 512], bf16, kind="ExternalInput").ap()
    bfar_d = nc.dram_tensor("bfar", [128, 4], f32, kind="ExternalInput").ap()
    bfarexp_d = nc.dram_tensor("bfarexp", [128, 4], f32,
                               kind="ExternalInput").ap()
    out_d = nc.dram_tensor("out", [B, S, D], f32, kind="ExternalOutput").ap()
    if DEBUG_DUMPS:
        dbg_qt = nc.dram_tensor("dbg_qt", [128, B * S], bf16,
                                kind="ExternalOutput").ap()
        dbg_kt = nc.dram_tensor("dbg_kt", [128, B * S], bf16,
                                kind="ExternalOutput").ap()
        dbg_v = nc.dram_tensor("dbg_v", [128, B * 16 * 130], bf16,
                               kind="ExternalOutput").ap()
        dbg_ut = nc.dram_tensor("dbg_ut", [128, 1024], bf16,
                                kind="ExternalOutput").ap()
        dbg_sct = nc.dram_tensor("dbg_sct", [128, 1024], f32,
                                 kind="ExternalOutput").ap()
        dbg_ctx = nc.dram_tensor("dbg_ctx", [2, 65, 512], f32,
                                 kind="ExternalOutput").ap()
        dbg_lct = nc.dram_tensor("dbg_lct", [2, 64, 512], bf16,
                                 kind="ExternalOutput").ap()
        dbg_rzb = nc.dram_tensor("dbg_rzb", [2, 64, 512], f32,
                                 kind="ExternalOutput").ap()

    VSLOT = 2 * 65  # [VA | 1 | VB | 1] per (b, kt)

    with tile.TileContext(nc) as tc, ExitStack() as ctx:
        const = ctx.enter_context(tc.tile_pool(name="const", bufs=1))

        xt_sb = const.tile([128, B * 8 * S], bf16, tag="xt")
        wq_sb = const.tile([128, 8 * 128], bf16, tag="wq")
        wk_sb = const.tile([128, 8 * 128], bf16, tag="wk")
        wv_sb = const.tile([128, 8 * 128], bf16, tag="wv")
        # small tensors first so the projection chains can start while the
        # bulk of xt is still in flight
        for w_sb, w_d in ((wq_sb, wqt_d), (wk_sb, wkt_d), (wv_sb, wvt_d)):
            for i in range(8):
                nc.sync.dma_start(w_sb[:, 128 * i : 128 * (i + 1)], w_d[i])
        wot_sb = const.tile([128, 1024], bf16, tag="wot")
        nc.sync.dma_start(wot_sb[:], wot_d[:])
        bfar_sb = const.tile([128, 4], f32, tag="bfar")
        nc.sync.dma_start(bfar_sb[:], bfar_d[:])
        bfarexp_sb = const.tile([128, 4], f32, tag="bfarexp")
        nc.sync.dma_start(bfarexp_sb[:], bfarexp_d[:])
        etab_sb = const.tile([128, 12 * 512], bf16, tag="etab")
        for i in range(12):
            nc.sync.dma_start(etab_sb[:, 512 * i : 512 * (i + 1)], etab_d[i])
        for i in range(B * 8):
            nc.sync.dma_start(xt_sb[:, S * i : S * (i + 1)], xt_d[i])
        ident = const.tile([128, 128], bf16, tag="ident")
        make_identity(nc, ident[:])

        qt_sb = const.tile([128, B * S], bf16, tag="qt")
        vt_sb = const.tile([128, B * S], bf16, tag="vtt")
        kt_sb = const.tile([128, B * S], bf16, tag="kt")
        v_sb = const.tile([128, B * N_KT * VSLOT], bf16, tag="v")
        for b in range(B):
            for kt in range(N_KT):
                base = (b * N_KT + kt) * VSLOT
                nc.gpsimd.memset(v_sb[:, base + 64 : base + 65], 1.0)
                nc.gpsimd.memset(v_sb[:, base + 129 : base + 130], 1.0)

        # ---- Phases B+C staggered ----
        # PE and ACT total work are nearly equal, but projections are
        # PE-only while attention is ACT-paced. Batch 0's projections run
        # first; batch 1's projections fill the PE during batch 0's first
        # attention block; then the two batches' q-blocks run offset by one
        # step. PSUM: sct 4 banks + ctx(b0) 2 + proj 2, and the proj pool is
        # swapped for ctx(b1)'s banks after step 0.
        def cls(m):
            if m <= -2:
                return 0  # far negative
            if m >= 5:
                return 1  # far positive
            return 2  # near

        with tc.tile_pool(name="scp", bufs=2, space="PSUM") as scp, \
             tc.tile_pool(name="ctxp0", bufs=1, space="PSUM") as ctxp0, \
             tc.tile_pool(name="utp", bufs=8) as utp, \
             tc.tile_pool(name="ostg", bufs=4) as ostg, \
             tc.tile_pool(name="nrm", bufs=1) as nrm:

            projp = tc.alloc_tile_pool(name="projp", bufs=2, space="PSUM")

            def emit_qk_chain(b, qb, wi):
                w_sb, dst = ((wq_sb, qt_sb), (wk_sb, kt_sb))[wi]
                ps = projp.tile([128, 512], f32, tag="proj",
                                name=f"pj_{b}_{qb}_{wi}")
                for dt in range(8):
                    nc.tensor.matmul(
                        ps[:],
                        lhsT=w_sb[:, 128 * dt : 128 * (dt + 1)],
                        rhs=xt_sb[:, (b * 8 + dt) * S + qb * QB :
                                  (b * 8 + dt) * S + qb * QB + QB],
                        start=(dt == 0), stop=(dt == 7),
                    )
                nc.vector.tensor_copy(
                    dst[:, b * S + qb * QB : b * S + qb * QB + QB], ps[:]
                )

            def emit_v_chain(b, st):
                ps = projp.tile([128, 128], f32, tag="proj",
                                name=f"vp_{b}_{st}")
                for dt in range(8):
                    nc.tensor.matmul(
                        ps[:],
                        lhsT=xt_sb[:, (b * 8 + dt) * S + st * KT :
                                   (b * 8 + dt) * S + st * KT + KT],
                        rhs=wv_sb[:, 128 * dt : 128 * (dt + 1)],
                        start=(dt == 0), stop=(dt == 7),
                    )
                base = (b * N_KT + st) * VSLOT
                nc.vector.tensor_copy(v_sb[:, base : base + 64], ps[:, 0:64])
                nc.vector.tensor_copy(v_sb[:, base + 65 : base + 129],
                                      ps[:, 64:128])

            def emit_stream_kp(kp, b, qb, ctxs):
                m0 = 2 * kp - 4 * qb
                m1 = m0 + 1
                c0, c1 = cls(m0), cls(m1)
                for hl in range(2):
                    sct = scp.tile([128, 1024], f32, tag="sc",
                                   name=f"sc_{qb}_{kp}_{b}_{hl}")
                    for half in range(2):
                        kt = 2 * kp + half
                        nc.tensor.matmul(
                            sct[:, 512 * half : 512 * (half + 1)],
                            lhsT=kt_sb[64 * hl : 64 * (hl + 1),
                                       b * S + kt * KT : b * S + kt * KT + KT],
                            rhs=qt_sb[64 * hl : 64 * (hl + 1),
                                      b * S + qb * QB : b * S + qb * QB + QB],
                            start=True, stop=True,
                        )
                    ut = utp.tile([128, 1024], bf16, tag="ut",
                                  name=f"ut_{qb}_{kp}_{b}_{hl}")
                    srcs = [(ut, 0), (ut, 512)]
                    if c0 == c1 and c0 != 2:
                        nc.scalar.activation(
                            ut[:], sct[:], Exp,
                            bias=bfar_sb[:, 2 * hl + c0 : 2 * hl + c0 + 1],
                            scale=SCALING,
                        )
                    else:
                        nc.scalar.activation(
                            ut[:], sct[:], Exp, bias=0.0, scale=SCALING
                        )
                        ut2 = utp.tile([128, 1024], bf16, tag="ut2",
                                       name=f"ut2_{qb}_{kp}_{b}_{hl}")
                        if c0 == c1 == 2:
                            ei = (hl * 6 + (m0 + 1)) * 512
                            nc.vector.tensor_mul(
                                ut2[:], ut[:], etab_sb[:, ei : ei + 1024]
                            )
                            srcs = [(ut2, 0), (ut2, 512)]
                        else:
                            for half, (m, cc) in enumerate(((m0, c0),
                                                           (m1, c1))):
                                usl = ut[:, 512 * half : 512 * (half + 1)]
                                osl = ut2[:, 512 * half : 512 * (half + 1)]
                                if cc == 2:
                                    ei = (hl * 6 + (m + 1)) * 512
                                    nc.vector.tensor_mul(
                                        osl, usl, etab_sb[:, ei : ei + 512]
                                    )
                                else:
                                    col = 2 * hl + cc
                                    nc.vector.tensor_scalar_mul(
                                        osl, usl, bfarexp_sb[:, col : col + 1]
                                    )
                                srcs[half] = (ut2, 512 * half)
                    for half in range(2):
                        kt = 2 * kp + half
                        base = (b * N_KT + kt) * VSLOT + 65 * hl
                        stile, soff = srcs[half]
                        nc.tensor.matmul(
                            ctxs[(b, hl)][:],
                            lhsT=v_sb[:, base : base + 65],
                            rhs=stile[:, soff : soff + 512],
                            start=(kt == 0), stop=(kt == N_KT - 1),
                        )

            def emit_norm_wo(b, qb, ctxs):
                lct = nrm.tile([128, 512], bf16, tag=f"lct{b}",
                               name=f"lct{b}_{qb}", bufs=2)
                for hl in range(2):
                    rz = nrm.tile([128, 512], f32, tag=f"rz{b}{hl}",
                                  name=f"rz{b}{hl}_{qb}")
                    nc.vector.tensor_copy(rz[0:1, :], ctxs[(b, hl)][64:65, :])
                    rzf = nrm.tile([128, 512], f32, tag=f"rzf{b}{hl}",
                                   name=f"rzf{b}{hl}_{qb}")
                    nc.vector.reciprocal_approx_fast(
                        out=rzf[0:1, :], in_=rz[0:1, :]
                    )
                    rzb = nrm.tile([64, 512], f32, tag=f"rzb{b}{hl}",
                                   name=f"rzb{b}{hl}_{qb}")
                    nc.gpsimd.partition_broadcast(
                        rzb[:], rzf[0:1, :], channels=64
                    )
                    nc.vector.tensor_mul(
                        lct[64 * hl : 64 * (hl + 1), :],
                        ctxs[(b, hl)][0:64, :], rzb[:])
                for st in range(4):
                    wo_ps = scp.tile([128, 1024], f32, tag="sc",
                                     name=f"wo_{qb}_{b}_{st}")
                    for nh in range(2):
                        nc.tensor.matmul(
                            wo_ps[:, nh * 512 : (nh + 1) * 512],
                            lhsT=lct[:, st * 128 : (st + 1) * 128],
                            rhs=wot_sb[:, nh * 512 : (nh + 1) * 512],
                            start=True, stop=True,
                        )
                    ot = ostg.tile([128, 1024], f32, tag="ot",
                                   name=f"ot_{qb}_{b}_{st}")
                    nc.vector.tensor_copy(ot[:], wo_ps[:])
                    srow = qb * QB + st * 128
                    nc.gpsimd.dma_start(out_d[b, srow : srow + 128, :], ot[:])

            # batch-0 projections
            for qb in range(N_QB):
                emit_qk_chain(0, qb, 0)
                emit_qk_chain(0, qb, 1)
                for st in (4 * qb, 4 * qb + 1, 4 * qb + 2, 4 * qb + 3):
                    emit_v_chain(0, st)

            # step 0: batch-0 qb0 attention + batch-1 projections as filler
            filler = []
            for qb in range(N_QB):
                filler.append(("qk", qb, 0))
                filler.append(("qk", qb, 1))
                for st in (4 * qb, 4 * qb + 1, 4 * qb + 2, 4 * qb + 3):
                    filler.append(("v", st))
            ctx_b0 = {}
            ctx_pool_of = {0: ctxp0}
            for hl in range(2):
                ctx_b0[(0, hl)] = ctxp0.tile([65, 512], f32, tag=f"c0{hl}",
                                             name=f"ctx0{hl}_0")
            for kp in range(8):
                emit_stream_kp(kp, 0, 0, ctx_b0)
                while filler and len(filler) > (7 - kp) * 3:
                    kind, *args = filler.pop(0)
                    if kind == "qk":
                        emit_qk_chain(1, args[0], args[1])
                    else:
                        emit_v_chain(1, args[0])

            projp.release()
            ctxp1 = tc.alloc_tile_pool(name="ctxp1", bufs=1, space="PSUM")

            # steps 1..4: offset schedule + pipelined norm/wo
            def alloc_ctx(b, qb):
                pool = ctxp0 if b == 0 else ctxp1
                d = {}
                for hl in range(2):
                    d[(b, hl)] = pool.tile([65, 512], f32, tag=f"c{b}{hl}",
                                           name=f"ctx{b}{hl}_{qb}")
                return d

            done_prev = [(0, 0, ctx_b0)]
            steps = [[(0, 1), (1, 0)], [(0, 2), (1, 1)], [(0, 3), (1, 2)],
                     [(1, 3)]]
            for streams in steps:
                ctxs_list = [(b, qb, alloc_ctx(b, qb)) for (b, qb) in streams]
                for kp in range(8):
                    for (b, qb, cd) in ctxs_list:
                        emit_stream_kp(kp, b, qb, cd)
                for (b, qb, cd) in done_prev:
                    emit_norm_wo(b, qb, cd)
                done_prev = ctxs_list
            for (b, qb, cd) in done_prev:
                emit_norm_wo(b, qb, cd)
            ctxp1.release()

        if DEBUG_DUMPS:
            nc.sync.dma_start(dbg_qt[:], qt_sb[:])
            nc.sync.dma_start(dbg_kt[:], kt_sb[:])
            nc.sync.dma_start(dbg_v[:], v_sb[:])

    nc.compile()
    _PROGRAM_CACHE["nc"] = nc
    return nc


def run(inputs, trace=False, trace_kwargs=None):
    """Returns (full_output, BassKernelResults)."""
    from concourse.bass_utils import run_bass_kernel_spmd

    hidden_states = np.asarray(inputs["hidden_states"], dtype=np.float32)
    Wq = np.asarray(inputs["Wq"], dtype=np.float32)
    Wk = np.asarray(inputs["Wk"], dtype=np.float32)
    Wv = np.asarray(inputs["Wv"], dtype=np.float32)
    Wo = np.asarray(inputs["Wo"], dtype=np.float32)
    rel_emb = np.asarray(inputs["rel_emb"], dtype=np.float32)

    xt = np.ascontiguousarray(hidden_states.transpose(0, 2, 1))  # [B, D, S]
    xt_bf16 = np.ascontiguousarray(xt.reshape(B * 8, 128, S)).astype(BF16)

    nc = _build_program()
    in_maps = [
        _prep_core_inputs(c, hidden_states, Wq, Wk, Wv, Wo, rel_emb, xt_bf16)
        for c in range(N_CORES)
    ]
    res = run_bass_kernel_spmd(
        nc, in_maps, list(range(N_CORES)), trace=trace,
        **(trace_kwargs or {}),
    )
    out = np.zeros((B, S, D), dtype=np.float32)
    for c in range(N_CORES):
        out += res.results[c]["out"]
    return out, res


def kernel(**inputs):
    out, _ = run(inputs)
    return out
